# revision 36
# baseline (speedup 1.0000x reference)
"""Trainium2 Bass kernel for the entropy-aware sampling model.

Contract: kernel(logits[4,128000] f32, attn_scores[32,4,32,1,4096] f32)
-> (samples_best [4,1] int32, scores [5] f32), matching the jax reference.

Distribution over 8 NeuronCores (one SPMD Bass program):
  - attn_scores sharded over the layer dim: core c gets layers [4c, 4c+4).
    Per layer, a [128, 4096] tile (partition = b*32+h):
      ScalarE Exp(+accum)      -> Z  = sum_s e^x          (softmax denom)
      VectorE fused mul-reduce -> T1 = sum_s x*e^x        (entropy numerator)
      PE matmul with the Z-scaled centering matrix (I - 0.25*same-head)
                               -> d = ap - mean_b(ap) in PSUM
      ScalarE Abs(+accum)      -> sum_s |d|               (agreement)
      GpSimd abs-reduce        -> sum_s |x|               (interaction strength)
  - logits sharded over vocab: core c gets half (c%2) of row (c//2):
      ScalarE Exp(+accum) -> E0; VectorE fused mul-reduces -> E1, E2
      (per-partition partial moments for logsumexp/entropy/varentropy)
      VectorE max8 -> per-partition top-8 candidates; the host takes the
      100th-largest candidate as a top-k threshold (a conservative bound:
      a candidate miss only lowers the threshold, growing the survivor
      set, never dropping a true top-k member).
Host: f64 merge of partials -> metrics -> exact f32 top-k/top-p/min-p
filter on the surviving logits, then jax.random.categorical (same two
lines as the reference, same environment/PRNG) for the 5 samples.
"""

import numpy as np

# Model constants.
LN2 = 0.6931471805599453
TEMP = 0.666
TOP_P = 0.9
TOP_K = 27
MIN_P = 0.03
N_SAMPLES = 5
ADA_TEMP_LOGITS = 0.3
ADA_TEMP_ATTN = 0.2
ADA_TEMP_AGREE = 0.2
ADA_TOP_P = 0.1
ADA_TOP_K_INT = 0.3
ADA_TOP_K_AGREE = 0.2
ADA_MIN_P = 0.5
SC_LE = 0.1
SC_AE = 0.2
SC_LV = 0.3
SC_AV = 0.4
SC_AG = 0.5
SC_IS = 0.6

B = 4
V = 128000
L = 32
H = 32
S = 4096
N_CORES = 8
NL = L // N_CORES          # layers per core
VH = V // 2                # logits half-row per core
LG_COLS = VH // 128        # 500

_CACHE = {}


def _build_bass():
    from concourse.bacc import Bacc
    import concourse.mybir as mybir
    from concourse.tile import TileContext
    from concourse.alu_op_type import AluOpType

    f32 = mybir.dt.float32
    bf16 = mybir.dt.bfloat16
    Exp = mybir.ActivationFunctionType.Exp
    Abs = mybir.ActivationFunctionType.Abs

    nc = Bacc()
    attn_in = nc.declare_dram_parameter("attn", [NL, 128, S], f32, isOutput=False)
    lg_in = nc.declare_dram_parameter("lg", [128, LG_COLS], f32, isOutput=False)
    pat_in = nc.declare_dram_parameter("pat", [128, 128], bf16, isOutput=False)
    stats_out = nc.declare_dram_parameter("stats", [128, 32], f32, isOutput=True)

    HS = S // 2  # half-layer columns (4 PSUM banks)

    with TileContext(nc) as tc:
        with (
            tc.tile_pool(name="big", bufs=4) as big,
            tc.tile_pool(name="ebuf", bufs=3) as ebuf,
            tc.tile_pool(name="abuf", bufs=2) as abuf,
            tc.tile_pool(name="junk", bufs=2) as junkp,
            tc.tile_pool(name="psum", bufs=2, space="PSUM") as psump,
            tc.tile_pool(name="small", bufs=1) as small,
            tc.tile_pool(name="sm2", bufs=2) as sm2,
        ):
            # Prefetch everything up front — attn layers first (critical path).
            # Full-layer transfers (16KB/partition rows = max descriptor size;
            # the DGE descriptor rate is the per-ring limit) alternating over
            # the two HWDGE rings (SP + ACT); small inputs go via SWDGE.
            # pat is tiny and feeds the per-layer critical chain — it must be
            # first in its FIFO ring, ahead of the multi-MB attention loads.
            pat = small.tile([128, 128], bf16)
            nc.sync.dma_start(out=pat, in_=pat_in[:])
            # The ACT-issued ring sustains ~2-3x the SP ring's rate here, so
            # it carries the three layers consumed first; the SP ring has
            # plenty of time to deliver the last layer.
            # Layer 0 gates everything: split it by partitions (keeps the
            # 16KB/row descriptors) across BOTH rings so its halves move
            # concurrently.
            ring = [None, nc.scalar, nc.scalar, nc.sync]
            atiles = []
            for l in range(NL):
                a = big.tile([128, S], f32, tag="a")
                if l == 0:
                    nc.scalar.dma_start(out=a[0:64, :], in_=attn_in[0, 0:64, :])
                    nc.sync.dma_start(out=a[64:128, :], in_=attn_in[0, 64:128, :])
                else:
                    ring[l].dma_start(out=a, in_=attn_in[l])
                atiles.append(a)
            # Warm the PE's activity monitor with junk matmuls while DMAs
            # stream — real matmuls then run at 2.4GHz instead of 1.2.
            warm = psump.tile([128, HS], f32, tag="d")
            for _ in range(24):
                nc.tensor.matmul(warm[:, 0:128], lhsT=pat, rhs=pat,
                                 start=True, stop=True)
            lgt = small.tile([128, LG_COLS], f32)
            nc.gpsimd.dma_start(out=lgt, in_=lg_in[:])

            zt = small.tile([128, NL], f32)
            t1t = small.tile([128, NL], f32)
            adt = small.tile([128, 2 * NL], f32)
            axt = small.tile([128, NL], f32)

            # The engines execute their queues in order and the scheduler's
            # cost model reorders badly here, so chain each engine's ops
            # explicitly (sync=False: ordering only, no extra semaphores).
            # Critical chain per layer: EXP -> recip/cmat -> matmuls -> ABS;
            # T1 (STT) and istr (reduce) are slack work interleaved on DVE.
            from bass_rust import add_dep_helper

            act_chain = []
            dve_chain = []

            def act(inst):
                if act_chain:
                    add_dep_helper(inst.ins, act_chain[-1].ins, sync=False,
                                   reason="act order")
                act_chain.append(inst)
                return inst

            def dve(inst):
                if dve_chain:
                    add_dep_helper(inst.ins, dve_chain[-1].ins, sync=False,
                                   reason="dve order")
                dve_chain.append(inst)
                return inst

            els = small.tile([128, 3], f32)

            def front(l):
                a = atiles[l]
                e = ebuf.tile([128, S], bf16, tag="e")
                act(nc.scalar.activation(
                    e, a, Exp, accum_out=zt[:, l : l + 1]))
                rz = sm2.tile([128, 1], f32, tag="rz")
                dve(nc.vector.reciprocal(rz, zt[:, l : l + 1]))
                cmat = sm2.tile([128, 128], bf16, tag="cmat")
                dve(nc.vector.tensor_scalar_mul(cmat, pat, rz))
                jt = junkp.tile([128, S], bf16, tag="jt")
                dve(nc.vector.scalar_tensor_tensor(
                    out=jt, in0=e, scalar=1.0, in1=a,
                    op0=AluOpType.mult, op1=AluOpType.mult,
                    accum_out=t1t[:, l : l + 1]))
                # istr reduce rides right behind: its data has long arrived.
                if l < 3:
                    dve(nc.vector.tensor_reduce(
                        axt[:, l : l + 1], a,
                        axis=mybir.AxisListType.X, op=AluOpType.add,
                        apply_absolute_value=True))
                ds = []
                for h in range(2):
                    d = psump.tile([128, HS], f32, tag="d")
                    for j in range(4):
                        nc.tensor.matmul(
                            d[:, j * 512 : (j + 1) * 512],
                            lhsT=cmat,
                            rhs=e[:, h * HS + j * 512 : h * HS + (j + 1) * 512],
                            start=True,
                            stop=True,
                        )
                    ds.append(d)
                return ds

            def back(l, ds):
                for h in range(2):
                    jt2 = junkp.tile([128, HS], bf16, tag="jt2")
                    act(nc.scalar.activation(
                        jt2, ds[h], Abs,
                        accum_out=adt[:, 2 * l + h : 2 * l + h + 1]))

            def logits_block():
                el = ebuf.tile([128, LG_COLS], f32, tag="el")
                act(nc.scalar.activation(el, lgt, Exp, accum_out=els[:, 0:1]))
                exl = junkp.tile([128, LG_COLS], f32, tag="exl")
                dve(nc.vector.scalar_tensor_tensor(
                    out=exl, in0=el, scalar=1.0, in1=lgt,
                    op0=AluOpType.mult, op1=AluOpType.mult,
                    accum_out=els[:, 1:2]))
                jl = junkp.tile([128, LG_COLS], f32, tag="jl")
                dve(nc.vector.scalar_tensor_tensor(
                    out=jl, in0=exl, scalar=1.0, in1=lgt,
                    op0=AluOpType.mult, op1=AluOpType.mult,
                    accum_out=els[:, 2:3]))
                cand = small.tile([128, 8], f32)
                dve(nc.vector.max(out=cand, in_=lgt))
                return cand

            prev = None
            cand = None
            for l in range(NL):
                ds = front(l)
                if prev is not None:
                    back(l - 1, prev)
                if l == 1:
                    cand = logits_block()
                prev = ds
            back(NL - 1, prev)
            # layer 3's istr on ACT (balances the engines' tails)
            jt3 = junkp.tile([128, S], bf16, tag="jt3")
            act(nc.scalar.activation(
                jt3, atiles[3], Abs, accum_out=axt[:, 3:4]))

            # Pack all stats into one tile -> single output DMA.
            stats = small.tile([128, 32], f32)
            nc.vector.tensor_copy(stats[:, 0:NL], zt)
            nc.vector.tensor_copy(stats[:, 4 : 4 + NL], t1t)
            nc.vector.tensor_copy(stats[:, 8 : 8 + 2 * NL], adt)
            nc.vector.tensor_copy(stats[:, 16 : 16 + NL], axt)
            nc.vector.tensor_copy(stats[:, 20:23], els)
            nc.vector.tensor_copy(stats[:, 23:31], cand)
            nc.sync.dma_start(out=stats_out[:], in_=stats)

    nc.finalize()
    return nc


def _get_nc():
    if "nc" not in _CACHE:
        _CACHE["nc"] = _build_bass()
    return _CACHE["nc"]


def _pattern():
    # Centering matrix pattern: delta(q,p) - 0.25 * [q mod 32 == p mod 32]
    # (partition order is (b, h), so same-head partitions are p ≡ h mod 32).
    # Values {1, 0.75, -0.25, 0} are exact in bf16.
    import ml_dtypes

    q = np.arange(128)
    pat = -0.25 * (q[:, None] % H == q[None, :] % H).astype(np.float32)
    pat[q, q] += 1.0
    return np.ascontiguousarray(pat.astype(ml_dtypes.bfloat16))


def _make_in_maps(logits, attn_scores):
    attn = np.ascontiguousarray(attn_scores.reshape(L, B * H, S).astype(np.float32))
    pat = _pattern()
    in_maps = []
    for c in range(N_CORES):
        m = {
            "attn": np.ascontiguousarray(attn[c * NL : (c + 1) * NL]),
            "lg": np.ascontiguousarray(
                logits[c // 2, (c % 2) * VH : (c % 2 + 1) * VH].reshape(128, LG_COLS)
            ),
            "pat": pat,
        }
        in_maps.append(m)
    return in_maps


def run_device(logits, attn_scores, trace=False, tmpdir=None):
    """Run the SPMD bass kernel; returns (per-core results list, BassKernelResults)."""
    from concourse.bass_utils import run_bass_kernel_spmd

    nc = _get_nc()
    in_maps = _make_in_maps(logits, attn_scores)
    res = run_bass_kernel_spmd(
        nc, in_maps, list(range(N_CORES)), trace=trace, tmpdir=tmpdir
    )
    return res.results, res


def _host_finish(logits, results):
    """Combine per-core device partials into the final samples/scores."""
    f32 = np.float32

    # ---- logits logsumexp / entropy / varentropy (f64 merge of partials) ----
    Z_row = np.zeros(B)
    S1_row = np.zeros(B)
    S2_row = np.zeros(B)
    for c in range(N_CORES):
        r = c // 2
        els = results[c]["stats"][:, 20:23].astype(np.float64)
        Z_row[r] += els[:, 0].sum()
        S1_row[r] += els[:, 1].sum()
        S2_row[r] += els[:, 2].sum()
    lse_row = np.log(Z_row)                      # log-sum-exp per row (ref 0)
    H_row = (lse_row - S1_row / Z_row) / LN2     # bits
    V_row = (S2_row / Z_row - (S1_row / Z_row) ** 2) / LN2**2
    logits_entropy = H_row.mean()
    logits_varentropy = V_row.mean()

    # ---- attention metrics ----
    aH = np.zeros((L, 128))                      # per (layer, b*32+h), bits
    agree_sum = 0.0
    istr_layers = np.zeros(L)
    for c in range(N_CORES):
        st = results[c]["stats"].astype(np.float64)
        z = st[:, 0:NL]
        t1 = st[:, 4 : 4 + NL]
        absd = st[:, 8 : 8 + 2 * NL]
        absx = st[:, 16 : 16 + NL]
        for li in range(NL):
            l = c * NL + li
            aH[l] = (np.log(z[:, li]) - t1[:, li] / z[:, li]) / LN2
            istr_layers[l] = absx[:, li].sum() / (B * H * S)
        agree_sum += absd.sum()
    attn_entropy = aH.mean()
    aH_bh = aH.reshape(L, B, H)
    aV = aH_bh.var(axis=2, ddof=1)               # [L, B]
    attn_varentropy = aV.mean()
    agreement = agree_sum / (L * B * H * S)
    interaction_strength = istr_layers.mean()

    # ---- adaptive parameters (mirror the reference's f32 scalar math) ----
    LE = f32(logits_entropy)
    LV = f32(logits_varentropy)
    AE = f32(attn_entropy)
    AV = f32(attn_varentropy)
    AG = f32(agreement)
    IS = f32(interaction_strength)
    lu = f32(LE + LV)
    au = f32(AE + AV)
    temperature = f32(
        f32(TEMP)
        * f32(
            f32(f32(1.0) + f32(f32(ADA_TEMP_LOGITS) * lu) + f32(f32(ADA_TEMP_ATTN) * au))
            - f32(f32(ADA_TEMP_AGREE) * AG)
        )
    )
    top_p = f32(np.clip(f32(f32(TOP_P) * f32(1.0 + f32(ADA_TOP_P) * AV)), 0.1, 1.0))
    top_k = int(
        np.clip(
            np.round(TOP_K * (1 + ADA_TOP_K_INT * float(IS) - ADA_TOP_K_AGREE * float(AG))),
            1,
            100,
        )
    )
    min_p = f32(np.clip(f32(f32(MIN_P) * f32(1.0 - f32(ADA_MIN_P) * lu)), 0.01, 0.5))

    # ---- top-k / top-p / min-p filter, exactly in f32 on the survivors ----
    # Device candidates -> conservative per-row threshold at the 100th largest.
    adj = np.full((B, V), -np.inf, dtype=np.float32)
    for r in range(B):
        cands = np.concatenate(
            [
                results[2 * r]["stats"][:, 23:31].ravel(),
                results[2 * r + 1]["stats"][:, 23:31].ravel(),
            ]
        )
        thr = np.sort(cands)[-100]               # <= true 100th largest value
        row = logits[r]
        idx = np.nonzero(row >= thr)[0]          # superset of the row's top-100
        scaled = (row[idx].astype(np.float32) / temperature).astype(np.float32)
        order = np.argsort(-scaled, kind="stable")
        sv = scaled[order]                       # descending, ties by index
        si = idx[order]
        # top-k: keep values >= kth largest (ties kept, like the reference)
        kth = sv[top_k - 1] if len(sv) >= top_k else sv[-1]
        keep = sv >= kth
        sv = sv[keep]
        si = si[keep]
        # top-p: softmax over survivors, cumulative mass, shifted mask
        m0 = sv[0]
        ex = np.exp((sv - m0).astype(np.float32)).astype(np.float32)
        p = (ex / ex.sum(dtype=np.float32)).astype(np.float32)
        cum = np.cumsum(p, dtype=np.float32)
        rm = np.zeros(len(sv), dtype=bool)
        rm[1:] = cum[:-1] > top_p
        sv = sv[~rm]
        si = si[~rm]
        # min-p on the re-normalized softmax
        ex = np.exp((sv - sv[0]).astype(np.float32)).astype(np.float32)
        p = (ex / ex.sum(dtype=np.float32)).astype(np.float32)
        keep = p >= min_p
        sv = sv[keep]
        si = si[keep]
        adj[r, si] = sv

    # ---- sampling: mirror the reference's jax.random calls exactly ----
    import jax
    import jax.numpy as jnp

    conf = f32(
        f32(f32(f32(1.0) - LE) * f32(SC_LE))
        + f32(f32(f32(1.0) - AE) * f32(SC_AE))
        + f32(f32(f32(1.0) - LV) * f32(SC_LV))
        + f32(f32(f32(1.0) - AV) * f32(SC_AV))
        + f32(AG * f32(SC_AG))
        + f32(IS * f32(SC_IS))
    )

    # The reference can only execute on the CPU backend in this container
    # (argsort is unsupported on trn2), and RBG PRNG bits are backend
    # specific — so draw the samples on CPU to match it bit-for-bit.
    cpu = jax.devices("cpu")[0]
    samples = []
    scores = np.zeros(N_SAMPLES, dtype=np.float32)
    with jax.default_device(cpu):
        adj_j = jnp.asarray(adj)
        key = jax.random.key(42)
        sampled = [
            np.asarray(
                jax.random.categorical(jax.random.fold_in(key, i), adj_j, axis=-1)
            ).astype(np.int32)[:, None]
            for i in range(N_SAMPLES)
        ]
    for i in range(N_SAMPLES):
        s = sampled[i]
        lsm_vals = (
            logits[np.arange(B), s[:, 0]].astype(np.float64) - lse_row
        ).astype(np.float32)
        log_prob = np.sum(lsm_vals, dtype=np.float32)
        samples.append(s)
        scores[i] = f32(log_prob + conf)
    best = int(np.argmax(scores))
    return samples[best], scores


def kernel(logits, attn_scores):
    logits = np.asarray(logits, dtype=np.float32)
    attn_scores = np.asarray(attn_scores, dtype=np.float32)
    results, _ = run_device(logits, attn_scores)
    return _host_finish(logits, results)


# revision 37
# speedup vs baseline: 1.1037x; 1.1037x over previous
"""Trainium2 Bass kernel for the entropy-aware sampling model.

Contract: kernel(logits[4,128000] f32, attn_scores[32,4,32,1,4096] f32)
-> (samples_best [4,1] int32, scores [5] f32), matching the jax reference.

Distribution over 8 NeuronCores (one SPMD Bass program):
  - attn_scores sharded over the layer dim: core c gets layers [4c, 4c+4).
    Per layer, a [128, 4096] tile (partition = b*32+h):
      ScalarE Exp(+accum)      -> Z  = sum_s e^x          (softmax denom)
      VectorE fused mul-reduce -> T1 = sum_s x*e^x        (entropy numerator)
      PE matmul with the Z-scaled centering matrix (I - 0.25*same-head)
                               -> d = ap - mean_b(ap) in PSUM
      ScalarE Abs(+accum)      -> sum_s |d|               (agreement)
      GpSimd abs-reduce        -> sum_s |x|               (interaction strength)
  - logits sharded over vocab: core c gets half (c%2) of row (c//2):
      ScalarE Exp(+accum) -> E0; VectorE fused mul-reduces -> E1, E2
      (per-partition partial moments for logsumexp/entropy/varentropy)
      VectorE max8 -> per-partition top-8 candidates; the host takes the
      100th-largest candidate as a top-k threshold (a conservative bound:
      a candidate miss only lowers the threshold, growing the survivor
      set, never dropping a true top-k member).
Host: f64 merge of partials -> metrics -> exact f32 top-k/top-p/min-p
filter on the surviving logits, then jax.random.categorical (same two
lines as the reference, same environment/PRNG) for the 5 samples.
"""

import numpy as np

# Model constants.
LN2 = 0.6931471805599453
TEMP = 0.666
TOP_P = 0.9
TOP_K = 27
MIN_P = 0.03
N_SAMPLES = 5
ADA_TEMP_LOGITS = 0.3
ADA_TEMP_ATTN = 0.2
ADA_TEMP_AGREE = 0.2
ADA_TOP_P = 0.1
ADA_TOP_K_INT = 0.3
ADA_TOP_K_AGREE = 0.2
ADA_MIN_P = 0.5
SC_LE = 0.1
SC_AE = 0.2
SC_LV = 0.3
SC_AV = 0.4
SC_AG = 0.5
SC_IS = 0.6

B = 4
V = 128000
L = 32
H = 32
S = 4096
N_CORES = 8
NL = L // N_CORES          # layers per core
VH = V // 2                # logits half-row per core
LG_COLS = VH // 128        # 500

_CACHE = {}


def _build_bass():
    from concourse.bacc import Bacc
    import concourse.mybir as mybir
    from concourse.tile import TileContext
    from concourse.alu_op_type import AluOpType

    f32 = mybir.dt.float32
    bf16 = mybir.dt.bfloat16
    Exp = mybir.ActivationFunctionType.Exp
    Abs = mybir.ActivationFunctionType.Abs

    nc = Bacc()
    attn_in = nc.declare_dram_parameter("attn", [NL, 128, S], f32, isOutput=False)
    lg_in = nc.declare_dram_parameter("lg", [128, LG_COLS], f32, isOutput=False)
    pat_in = nc.declare_dram_parameter("pat", [128, 128], bf16, isOutput=False)
    stats_out = nc.declare_dram_parameter("stats", [128, 32], f32, isOutput=True)

    HS = S // 2  # half-layer columns (4 PSUM banks)

    with TileContext(nc) as tc:
        with (
            tc.tile_pool(name="big", bufs=4) as big,
            tc.tile_pool(name="ebuf", bufs=3) as ebuf,
            tc.tile_pool(name="abuf", bufs=2) as abuf,
            tc.tile_pool(name="junk", bufs=2) as junkp,
            tc.tile_pool(name="psum", bufs=2, space="PSUM") as psump,
            tc.tile_pool(name="small", bufs=1) as small,
            tc.tile_pool(name="sm2", bufs=2) as sm2,
        ):
            # Prefetch everything up front — attn layers first (critical path).
            # Full-layer transfers (16KB/partition rows = max descriptor size;
            # the DGE descriptor rate is the per-ring limit) alternating over
            # the two HWDGE rings (SP + ACT); small inputs go via SWDGE.
            # pat is tiny and feeds the per-layer critical chain — it must be
            # first in its FIFO ring, ahead of the multi-MB attention loads.
            pat = small.tile([128, 128], bf16)
            nc.sync.dma_start(out=pat, in_=pat_in[:])
            # The ACT-issued ring sustains ~2-3x the SP ring's rate here, so
            # it carries the three layers consumed first; the SP ring has
            # plenty of time to deliver the last layer.
            ring = [nc.scalar, nc.scalar, nc.scalar, nc.sync]
            atiles = []
            for l in range(NL):
                a = big.tile([128, S], f32, tag="a")
                ring[l].dma_start(out=a, in_=attn_in[l])
                atiles.append(a)
            lgt = small.tile([128, LG_COLS], f32)
            nc.gpsimd.dma_start(out=lgt, in_=lg_in[:])

            zt = small.tile([128, NL], f32)
            t1t = small.tile([128, NL], f32)
            adt = small.tile([128, 2 * NL], f32)
            axt = small.tile([128, NL], f32)

            # The engines execute their queues in order and the scheduler's
            # cost model reorders badly here, so chain each engine's ops
            # explicitly (sync=False: ordering only, no extra semaphores).
            # Critical chain per layer: EXP -> recip/cmat -> matmuls -> ABS;
            # T1 (STT) and istr (reduce) are slack work interleaved on DVE.
            from bass_rust import add_dep_helper

            act_chain = []
            dve_chain = []

            def act(inst):
                if act_chain:
                    add_dep_helper(inst.ins, act_chain[-1].ins, sync=False,
                                   reason="act order")
                act_chain.append(inst)
                return inst

            def dve(inst):
                if dve_chain:
                    add_dep_helper(inst.ins, dve_chain[-1].ins, sync=False,
                                   reason="dve order")
                dve_chain.append(inst)
                return inst

            els = small.tile([128, 3], f32)

            def front(l):
                a = atiles[l]
                e = ebuf.tile([128, S], bf16, tag="e")
                act(nc.scalar.activation(
                    e, a, Exp, accum_out=zt[:, l : l + 1]))
                rz = sm2.tile([128, 1], f32, tag="rz")
                dve(nc.vector.reciprocal(rz, zt[:, l : l + 1]))
                cmat = sm2.tile([128, 128], bf16, tag="cmat")
                dve(nc.vector.tensor_scalar_mul(cmat, pat, rz))
                jt = junkp.tile([128, S], bf16, tag="jt")
                dve(nc.vector.scalar_tensor_tensor(
                    out=jt, in0=e, scalar=1.0, in1=a,
                    op0=AluOpType.mult, op1=AluOpType.mult,
                    accum_out=t1t[:, l : l + 1]))
                # istr reduce rides right behind: its data has long arrived.
                if l < 3:
                    dve(nc.vector.tensor_reduce(
                        axt[:, l : l + 1], a,
                        axis=mybir.AxisListType.X, op=AluOpType.add,
                        apply_absolute_value=True))
                ds = []
                for h in range(2):
                    d = psump.tile([128, HS], f32, tag="d")
                    for j in range(4):
                        nc.tensor.matmul(
                            d[:, j * 512 : (j + 1) * 512],
                            lhsT=cmat,
                            rhs=e[:, h * HS + j * 512 : h * HS + (j + 1) * 512],
                            start=True,
                            stop=True,
                        )
                    ds.append(d)
                return ds

            def back(l, ds):
                for h in range(2):
                    jt2 = junkp.tile([128, HS], bf16, tag="jt2")
                    act(nc.scalar.activation(
                        jt2, ds[h], Abs,
                        accum_out=adt[:, 2 * l + h : 2 * l + h + 1]))

            def logits_block():
                el = ebuf.tile([128, LG_COLS], f32, tag="el")
                act(nc.scalar.activation(el, lgt, Exp, accum_out=els[:, 0:1]))
                exl = junkp.tile([128, LG_COLS], f32, tag="exl")
                dve(nc.vector.scalar_tensor_tensor(
                    out=exl, in0=el, scalar=1.0, in1=lgt,
                    op0=AluOpType.mult, op1=AluOpType.mult,
                    accum_out=els[:, 1:2]))
                jl = junkp.tile([128, LG_COLS], f32, tag="jl")
                dve(nc.vector.scalar_tensor_tensor(
                    out=jl, in0=exl, scalar=1.0, in1=lgt,
                    op0=AluOpType.mult, op1=AluOpType.mult,
                    accum_out=els[:, 2:3]))
                cand = small.tile([128, 8], f32)
                dve(nc.vector.max(out=cand, in_=lgt))
                return cand

            prev = None
            cand = None
            for l in range(NL):
                ds = front(l)
                if prev is not None:
                    back(l - 1, prev)
                if l == 1:
                    cand = logits_block()
                prev = ds
            back(NL - 1, prev)
            # layer 3's istr on ACT (balances the engines' tails)
            jt3 = junkp.tile([128, S], bf16, tag="jt3")
            act(nc.scalar.activation(
                jt3, atiles[3], Abs, accum_out=axt[:, 3:4]))

            # Pack all stats into one tile -> single output DMA.
            stats = small.tile([128, 32], f32)
            nc.vector.tensor_copy(stats[:, 0:NL], zt)
            nc.vector.tensor_copy(stats[:, 4 : 4 + NL], t1t)
            nc.vector.tensor_copy(stats[:, 8 : 8 + 2 * NL], adt)
            nc.vector.tensor_copy(stats[:, 16 : 16 + NL], axt)
            nc.vector.tensor_copy(stats[:, 20:23], els)
            nc.vector.tensor_copy(stats[:, 23:31], cand)
            nc.sync.dma_start(out=stats_out[:], in_=stats)

    nc.finalize()
    return nc


def _get_nc():
    if "nc" not in _CACHE:
        _CACHE["nc"] = _build_bass()
    return _CACHE["nc"]


def _pattern():
    # Centering matrix pattern: delta(q,p) - 0.25 * [q mod 32 == p mod 32]
    # (partition order is (b, h), so same-head partitions are p ≡ h mod 32).
    # Values {1, 0.75, -0.25, 0} are exact in bf16.
    import ml_dtypes

    q = np.arange(128)
    pat = -0.25 * (q[:, None] % H == q[None, :] % H).astype(np.float32)
    pat[q, q] += 1.0
    return np.ascontiguousarray(pat.astype(ml_dtypes.bfloat16))


def _make_in_maps(logits, attn_scores):
    attn = np.ascontiguousarray(attn_scores.reshape(L, B * H, S).astype(np.float32))
    pat = _pattern()
    in_maps = []
    for c in range(N_CORES):
        m = {
            "attn": np.ascontiguousarray(attn[c * NL : (c + 1) * NL]),
            "lg": np.ascontiguousarray(
                logits[c // 2, (c % 2) * VH : (c % 2 + 1) * VH].reshape(128, LG_COLS)
            ),
            "pat": pat,
        }
        in_maps.append(m)
    return in_maps


def run_device(logits, attn_scores, trace=False, tmpdir=None):
    """Run the SPMD bass kernel; returns (per-core results list, BassKernelResults)."""
    from concourse.bass_utils import run_bass_kernel_spmd

    nc = _get_nc()
    in_maps = _make_in_maps(logits, attn_scores)
    res = run_bass_kernel_spmd(
        nc, in_maps, list(range(N_CORES)), trace=trace, tmpdir=tmpdir
    )
    return res.results, res


def _host_finish(logits, results):
    """Combine per-core device partials into the final samples/scores."""
    f32 = np.float32

    # ---- logits logsumexp / entropy / varentropy (f64 merge of partials) ----
    Z_row = np.zeros(B)
    S1_row = np.zeros(B)
    S2_row = np.zeros(B)
    for c in range(N_CORES):
        r = c // 2
        els = results[c]["stats"][:, 20:23].astype(np.float64)
        Z_row[r] += els[:, 0].sum()
        S1_row[r] += els[:, 1].sum()
        S2_row[r] += els[:, 2].sum()
    lse_row = np.log(Z_row)                      # log-sum-exp per row (ref 0)
    H_row = (lse_row - S1_row / Z_row) / LN2     # bits
    V_row = (S2_row / Z_row - (S1_row / Z_row) ** 2) / LN2**2
    logits_entropy = H_row.mean()
    logits_varentropy = V_row.mean()

    # ---- attention metrics ----
    aH = np.zeros((L, 128))                      # per (layer, b*32+h), bits
    agree_sum = 0.0
    istr_layers = np.zeros(L)
    for c in range(N_CORES):
        st = results[c]["stats"].astype(np.float64)
        z = st[:, 0:NL]
        t1 = st[:, 4 : 4 + NL]
        absd = st[:, 8 : 8 + 2 * NL]
        absx = st[:, 16 : 16 + NL]
        for li in range(NL):
            l = c * NL + li
            aH[l] = (np.log(z[:, li]) - t1[:, li] / z[:, li]) / LN2
            istr_layers[l] = absx[:, li].sum() / (B * H * S)
        agree_sum += absd.sum()
    attn_entropy = aH.mean()
    aH_bh = aH.reshape(L, B, H)
    aV = aH_bh.var(axis=2, ddof=1)               # [L, B]
    attn_varentropy = aV.mean()
    agreement = agree_sum / (L * B * H * S)
    interaction_strength = istr_layers.mean()

    # ---- adaptive parameters (mirror the reference's f32 scalar math) ----
    LE = f32(logits_entropy)
    LV = f32(logits_varentropy)
    AE = f32(attn_entropy)
    AV = f32(attn_varentropy)
    AG = f32(agreement)
    IS = f32(interaction_strength)
    lu = f32(LE + LV)
    au = f32(AE + AV)
    temperature = f32(
        f32(TEMP)
        * f32(
            f32(f32(1.0) + f32(f32(ADA_TEMP_LOGITS) * lu) + f32(f32(ADA_TEMP_ATTN) * au))
            - f32(f32(ADA_TEMP_AGREE) * AG)
        )
    )
    top_p = f32(np.clip(f32(f32(TOP_P) * f32(1.0 + f32(ADA_TOP_P) * AV)), 0.1, 1.0))
    top_k = int(
        np.clip(
            np.round(TOP_K * (1 + ADA_TOP_K_INT * float(IS) - ADA_TOP_K_AGREE * float(AG))),
            1,
            100,
        )
    )
    min_p = f32(np.clip(f32(f32(MIN_P) * f32(1.0 - f32(ADA_MIN_P) * lu)), 0.01, 0.5))

    # ---- top-k / top-p / min-p filter, exactly in f32 on the survivors ----
    # Device candidates -> conservative per-row threshold at the 100th largest.
    adj = np.full((B, V), -np.inf, dtype=np.float32)
    for r in range(B):
        cands = np.concatenate(
            [
                results[2 * r]["stats"][:, 23:31].ravel(),
                results[2 * r + 1]["stats"][:, 23:31].ravel(),
            ]
        )
        thr = np.sort(cands)[-100]               # <= true 100th largest value
        row = logits[r]
        idx = np.nonzero(row >= thr)[0]          # superset of the row's top-100
        scaled = (row[idx].astype(np.float32) / temperature).astype(np.float32)
        order = np.argsort(-scaled, kind="stable")
        sv = scaled[order]                       # descending, ties by index
        si = idx[order]
        # top-k: keep values >= kth largest (ties kept, like the reference)
        kth = sv[top_k - 1] if len(sv) >= top_k else sv[-1]
        keep = sv >= kth
        sv = sv[keep]
        si = si[keep]
        # top-p: softmax over survivors, cumulative mass, shifted mask
        m0 = sv[0]
        ex = np.exp((sv - m0).astype(np.float32)).astype(np.float32)
        p = (ex / ex.sum(dtype=np.float32)).astype(np.float32)
        cum = np.cumsum(p, dtype=np.float32)
        rm = np.zeros(len(sv), dtype=bool)
        rm[1:] = cum[:-1] > top_p
        sv = sv[~rm]
        si = si[~rm]
        # min-p on the re-normalized softmax
        ex = np.exp((sv - sv[0]).astype(np.float32)).astype(np.float32)
        p = (ex / ex.sum(dtype=np.float32)).astype(np.float32)
        keep = p >= min_p
        sv = sv[keep]
        si = si[keep]
        adj[r, si] = sv

    # ---- sampling: mirror the reference's jax.random calls exactly ----
    import jax
    import jax.numpy as jnp

    conf = f32(
        f32(f32(f32(1.0) - LE) * f32(SC_LE))
        + f32(f32(f32(1.0) - AE) * f32(SC_AE))
        + f32(f32(f32(1.0) - LV) * f32(SC_LV))
        + f32(f32(f32(1.0) - AV) * f32(SC_AV))
        + f32(AG * f32(SC_AG))
        + f32(IS * f32(SC_IS))
    )

    # The reference can only execute on the CPU backend in this container
    # (argsort is unsupported on trn2), and RBG PRNG bits are backend
    # specific — so draw the samples on CPU to match it bit-for-bit.
    cpu = jax.devices("cpu")[0]
    samples = []
    scores = np.zeros(N_SAMPLES, dtype=np.float32)
    with jax.default_device(cpu):
        adj_j = jnp.asarray(adj)
        key = jax.random.key(42)
        sampled = [
            np.asarray(
                jax.random.categorical(jax.random.fold_in(key, i), adj_j, axis=-1)
            ).astype(np.int32)[:, None]
            for i in range(N_SAMPLES)
        ]
    for i in range(N_SAMPLES):
        s = sampled[i]
        lsm_vals = (
            logits[np.arange(B), s[:, 0]].astype(np.float64) - lse_row
        ).astype(np.float32)
        log_prob = np.sum(lsm_vals, dtype=np.float32)
        samples.append(s)
        scores[i] = f32(log_prob + conf)
    best = int(np.argmax(scores))
    return samples[best], scores


def kernel(logits, attn_scores):
    logits = np.asarray(logits, dtype=np.float32)
    attn_scores = np.asarray(attn_scores, dtype=np.float32)
    results, _ = run_device(logits, attn_scores)
    return _host_finish(logits, results)


# revision 38
# speedup vs baseline: 1.1750x; 1.0646x over previous
"""Trainium2 Bass kernel for the entropy-aware sampling model.

Contract: kernel(logits[4,128000] f32, attn_scores[32,4,32,1,4096] f32)
-> (samples_best [4,1] int32, scores [5] f32), matching the jax reference.

Distribution over 8 NeuronCores (one SPMD Bass program):
  - attn_scores sharded over the layer dim: core c gets layers [4c, 4c+4).
    Per layer, a [128, 4096] tile (partition = b*32+h):
      ScalarE Exp(+accum)      -> Z  = sum_s e^x          (softmax denom)
      VectorE fused mul-reduce -> T1 = sum_s x*e^x        (entropy numerator)
      PE matmul with the Z-scaled centering matrix (I - 0.25*same-head)
                               -> d = ap - mean_b(ap) in PSUM
      ScalarE Abs(+accum)      -> sum_s |d|               (agreement)
      GpSimd abs-reduce        -> sum_s |x|               (interaction strength)
  - logits sharded over vocab: core c gets half (c%2) of row (c//2):
      ScalarE Exp(+accum) -> E0; VectorE fused mul-reduces -> E1, E2
      (per-partition partial moments for logsumexp/entropy/varentropy)
      VectorE max8 -> per-partition top-8 candidates; the host takes the
      100th-largest candidate as a top-k threshold (a conservative bound:
      a candidate miss only lowers the threshold, growing the survivor
      set, never dropping a true top-k member).
Host: f64 merge of partials -> metrics -> exact f32 top-k/top-p/min-p
filter on the surviving logits, then jax.random.categorical (same two
lines as the reference, same environment/PRNG) for the 5 samples.
"""

import numpy as np

# Model constants.
LN2 = 0.6931471805599453
TEMP = 0.666
TOP_P = 0.9
TOP_K = 27
MIN_P = 0.03
N_SAMPLES = 5
ADA_TEMP_LOGITS = 0.3
ADA_TEMP_ATTN = 0.2
ADA_TEMP_AGREE = 0.2
ADA_TOP_P = 0.1
ADA_TOP_K_INT = 0.3
ADA_TOP_K_AGREE = 0.2
ADA_MIN_P = 0.5
SC_LE = 0.1
SC_AE = 0.2
SC_LV = 0.3
SC_AV = 0.4
SC_AG = 0.5
SC_IS = 0.6

B = 4
V = 128000
L = 32
H = 32
S = 4096
N_CORES = 8
NL = L // N_CORES          # layers per core
VH = V // 2                # logits half-row per core
LG_COLS = VH // 128        # 500

_CACHE = {}


def _build_bass():
    from concourse.bacc import Bacc
    import concourse.mybir as mybir
    from concourse.tile import TileContext
    from concourse.alu_op_type import AluOpType

    f32 = mybir.dt.float32
    bf16 = mybir.dt.bfloat16
    Exp = mybir.ActivationFunctionType.Exp
    Abs = mybir.ActivationFunctionType.Abs

    nc = Bacc()
    attn_in = nc.declare_dram_parameter("attn", [NL, 128, S], f32, isOutput=False)
    lg_in = nc.declare_dram_parameter("lg", [128, LG_COLS], f32, isOutput=False)
    pat_in = nc.declare_dram_parameter("pat", [128, 128], bf16, isOutput=False)
    stats_out = nc.declare_dram_parameter("stats", [128, 32], f32, isOutput=True)

    HS = S // 2  # half-layer columns (4 PSUM banks)

    with TileContext(nc) as tc:
        with (
            tc.tile_pool(name="big", bufs=4) as big,
            tc.tile_pool(name="ebuf", bufs=4) as ebuf,
            tc.tile_pool(name="abuf", bufs=2) as abuf,
            tc.tile_pool(name="junk", bufs=2) as junkp,
            tc.tile_pool(name="psum", bufs=2, space="PSUM") as psump,
            tc.tile_pool(name="small", bufs=1) as small,
            tc.tile_pool(name="sm2", bufs=2) as sm2,
        ):
            # Prefetch everything up front — attn layers first (critical path).
            # Full-layer transfers (16KB/partition rows = max descriptor size;
            # the DGE descriptor rate is the per-ring limit) alternating over
            # the two HWDGE rings (SP + ACT); small inputs go via SWDGE.
            # pat is tiny and feeds the per-layer critical chain — it must be
            # first in its FIFO ring, ahead of the multi-MB attention loads.
            pat = small.tile([128, 128], bf16)
            nc.sync.dma_start(out=pat, in_=pat_in[:])
            # The ACT-issued ring sustains ~2-3x the SP ring's rate here, so
            # it carries the three layers consumed first; the SP ring has
            # plenty of time to deliver the last layer.
            ring = [nc.scalar, nc.scalar, nc.scalar, nc.sync]
            atiles = []
            for l in range(NL):
                a = big.tile([128, S], f32, tag="a")
                ring[l].dma_start(out=a, in_=attn_in[l])
                atiles.append(a)
            lgt = small.tile([128, LG_COLS], f32)
            nc.gpsimd.dma_start(out=lgt, in_=lg_in[:])

            zt = small.tile([128, NL], f32)
            t1t = small.tile([128, NL], f32)
            adt = small.tile([128, 2 * NL], f32)
            axt = small.tile([128, NL], f32)

            # The engines execute their queues in order and the scheduler's
            # cost model reorders badly here, so chain each engine's ops
            # explicitly (sync=False: ordering only, no extra semaphores).
            # Critical chain per layer: EXP -> recip/cmat -> matmuls -> ABS;
            # T1 (STT) and istr (reduce) are slack work interleaved on DVE.
            from bass_rust import add_dep_helper

            act_chain = []
            dve_chain = []

            def act(inst):
                if act_chain:
                    add_dep_helper(inst.ins, act_chain[-1].ins, sync=False,
                                   reason="act order")
                act_chain.append(inst)
                return inst

            def dve(inst):
                if dve_chain:
                    add_dep_helper(inst.ins, dve_chain[-1].ins, sync=False,
                                   reason="dve order")
                dve_chain.append(inst)
                return inst

            els = small.tile([128, 3], f32)

            def front(l):
                a = atiles[l]
                e = ebuf.tile([128, S], bf16, tag="e")
                act(nc.scalar.activation(
                    e, a, Exp, accum_out=zt[:, l : l + 1]))
                # istr reduce first: it needs only the DMA'd tile, so it
                # fills the DVE while EXP computes Z for this layer.
                if l < 3:
                    dve(nc.vector.tensor_reduce(
                        axt[:, l : l + 1], a,
                        axis=mybir.AxisListType.X, op=AluOpType.add,
                        apply_absolute_value=True))
                rz = sm2.tile([128, 1], f32, tag="rz")
                dve(nc.vector.reciprocal(rz, zt[:, l : l + 1]))
                cmat = sm2.tile([128, 128], bf16, tag="cmat")
                dve(nc.vector.tensor_scalar_mul(cmat, pat, rz))
                jt = junkp.tile([128, S], bf16, tag="jt")
                dve(nc.vector.scalar_tensor_tensor(
                    out=jt, in0=e, scalar=1.0, in1=a,
                    op0=AluOpType.mult, op1=AluOpType.mult,
                    accum_out=t1t[:, l : l + 1]))
                ds = []
                for h in range(2):
                    d = psump.tile([128, HS], f32, tag="d")
                    for j in range(4):
                        nc.tensor.matmul(
                            d[:, j * 512 : (j + 1) * 512],
                            lhsT=cmat,
                            rhs=e[:, h * HS + j * 512 : h * HS + (j + 1) * 512],
                            start=True,
                            stop=True,
                        )
                    ds.append(d)
                return ds

            def back(l, ds):
                for h in range(2):
                    jt2 = junkp.tile([128, HS], bf16, tag="jt2")
                    act(nc.scalar.activation(
                        jt2, ds[h], Abs,
                        accum_out=adt[:, 2 * l + h : 2 * l + h + 1]))

            def logits_block():
                el = ebuf.tile([128, LG_COLS], f32, tag="el")
                act(nc.scalar.activation(el, lgt, Exp, accum_out=els[:, 0:1]))
                exl = junkp.tile([128, LG_COLS], f32, tag="exl")
                dve(nc.vector.scalar_tensor_tensor(
                    out=exl, in0=el, scalar=1.0, in1=lgt,
                    op0=AluOpType.mult, op1=AluOpType.mult,
                    accum_out=els[:, 1:2]))
                jl = junkp.tile([128, LG_COLS], f32, tag="jl")
                dve(nc.vector.scalar_tensor_tensor(
                    out=jl, in0=exl, scalar=1.0, in1=lgt,
                    op0=AluOpType.mult, op1=AluOpType.mult,
                    accum_out=els[:, 2:3]))
                cand = small.tile([128, 8], f32)
                dve(nc.vector.max(out=cand, in_=lgt))
                return cand

            prev = None
            cand = None
            for l in range(NL):
                ds = front(l)
                if prev is not None:
                    back(l - 1, prev)
                if l == 1:
                    cand = logits_block()
                prev = ds
            back(NL - 1, prev)
            # layer 3's istr on ACT (balances the engines' tails)
            jt3 = junkp.tile([128, S], bf16, tag="jt3")
            act(nc.scalar.activation(
                jt3, atiles[3], Abs, accum_out=axt[:, 3:4]))

            # Pack all stats into one tile -> single output DMA.
            stats = small.tile([128, 32], f32)
            nc.vector.tensor_copy(stats[:, 0:NL], zt)
            nc.vector.tensor_copy(stats[:, 4 : 4 + NL], t1t)
            nc.vector.tensor_copy(stats[:, 8 : 8 + 2 * NL], adt)
            nc.vector.tensor_copy(stats[:, 16 : 16 + NL], axt)
            nc.vector.tensor_copy(stats[:, 20:23], els)
            nc.vector.tensor_copy(stats[:, 23:31], cand)
            nc.sync.dma_start(out=stats_out[:], in_=stats)

    nc.finalize()
    return nc


def _get_nc():
    if "nc" not in _CACHE:
        _CACHE["nc"] = _build_bass()
    return _CACHE["nc"]


def _pattern():
    # Centering matrix pattern: delta(q,p) - 0.25 * [q mod 32 == p mod 32]
    # (partition order is (b, h), so same-head partitions are p ≡ h mod 32).
    # Values {1, 0.75, -0.25, 0} are exact in bf16.
    import ml_dtypes

    q = np.arange(128)
    pat = -0.25 * (q[:, None] % H == q[None, :] % H).astype(np.float32)
    pat[q, q] += 1.0
    return np.ascontiguousarray(pat.astype(ml_dtypes.bfloat16))


def _make_in_maps(logits, attn_scores):
    attn = np.ascontiguousarray(attn_scores.reshape(L, B * H, S).astype(np.float32))
    pat = _pattern()
    in_maps = []
    for c in range(N_CORES):
        m = {
            "attn": np.ascontiguousarray(attn[c * NL : (c + 1) * NL]),
            "lg": np.ascontiguousarray(
                logits[c // 2, (c % 2) * VH : (c % 2 + 1) * VH].reshape(128, LG_COLS)
            ),
            "pat": pat,
        }
        in_maps.append(m)
    return in_maps


def run_device(logits, attn_scores, trace=False, tmpdir=None):
    """Run the SPMD bass kernel; returns (per-core results list, BassKernelResults)."""
    from concourse.bass_utils import run_bass_kernel_spmd

    nc = _get_nc()
    in_maps = _make_in_maps(logits, attn_scores)
    res = run_bass_kernel_spmd(
        nc, in_maps, list(range(N_CORES)), trace=trace, tmpdir=tmpdir
    )
    return res.results, res


def _host_finish(logits, results):
    """Combine per-core device partials into the final samples/scores."""
    f32 = np.float32

    # ---- logits logsumexp / entropy / varentropy (f64 merge of partials) ----
    Z_row = np.zeros(B)
    S1_row = np.zeros(B)
    S2_row = np.zeros(B)
    for c in range(N_CORES):
        r = c // 2
        els = results[c]["stats"][:, 20:23].astype(np.float64)
        Z_row[r] += els[:, 0].sum()
        S1_row[r] += els[:, 1].sum()
        S2_row[r] += els[:, 2].sum()
    lse_row = np.log(Z_row)                      # log-sum-exp per row (ref 0)
    H_row = (lse_row - S1_row / Z_row) / LN2     # bits
    V_row = (S2_row / Z_row - (S1_row / Z_row) ** 2) / LN2**2
    logits_entropy = H_row.mean()
    logits_varentropy = V_row.mean()

    # ---- attention metrics ----
    aH = np.zeros((L, 128))                      # per (layer, b*32+h), bits
    agree_sum = 0.0
    istr_layers = np.zeros(L)
    for c in range(N_CORES):
        st = results[c]["stats"].astype(np.float64)
        z = st[:, 0:NL]
        t1 = st[:, 4 : 4 + NL]
        absd = st[:, 8 : 8 + 2 * NL]
        absx = st[:, 16 : 16 + NL]
        for li in range(NL):
            l = c * NL + li
            aH[l] = (np.log(z[:, li]) - t1[:, li] / z[:, li]) / LN2
            istr_layers[l] = absx[:, li].sum() / (B * H * S)
        agree_sum += absd.sum()
    attn_entropy = aH.mean()
    aH_bh = aH.reshape(L, B, H)
    aV = aH_bh.var(axis=2, ddof=1)               # [L, B]
    attn_varentropy = aV.mean()
    agreement = agree_sum / (L * B * H * S)
    interaction_strength = istr_layers.mean()

    # ---- adaptive parameters (mirror the reference's f32 scalar math) ----
    LE = f32(logits_entropy)
    LV = f32(logits_varentropy)
    AE = f32(attn_entropy)
    AV = f32(attn_varentropy)
    AG = f32(agreement)
    IS = f32(interaction_strength)
    lu = f32(LE + LV)
    au = f32(AE + AV)
    temperature = f32(
        f32(TEMP)
        * f32(
            f32(f32(1.0) + f32(f32(ADA_TEMP_LOGITS) * lu) + f32(f32(ADA_TEMP_ATTN) * au))
            - f32(f32(ADA_TEMP_AGREE) * AG)
        )
    )
    top_p = f32(np.clip(f32(f32(TOP_P) * f32(1.0 + f32(ADA_TOP_P) * AV)), 0.1, 1.0))
    top_k = int(
        np.clip(
            np.round(TOP_K * (1 + ADA_TOP_K_INT * float(IS) - ADA_TOP_K_AGREE * float(AG))),
            1,
            100,
        )
    )
    min_p = f32(np.clip(f32(f32(MIN_P) * f32(1.0 - f32(ADA_MIN_P) * lu)), 0.01, 0.5))

    # ---- top-k / top-p / min-p filter, exactly in f32 on the survivors ----
    # Device candidates -> conservative per-row threshold at the 100th largest.
    adj = np.full((B, V), -np.inf, dtype=np.float32)
    for r in range(B):
        cands = np.concatenate(
            [
                results[2 * r]["stats"][:, 23:31].ravel(),
                results[2 * r + 1]["stats"][:, 23:31].ravel(),
            ]
        )
        thr = np.sort(cands)[-100]               # <= true 100th largest value
        row = logits[r]
        idx = np.nonzero(row >= thr)[0]          # superset of the row's top-100
        scaled = (row[idx].astype(np.float32) / temperature).astype(np.float32)
        order = np.argsort(-scaled, kind="stable")
        sv = scaled[order]                       # descending, ties by index
        si = idx[order]
        # top-k: keep values >= kth largest (ties kept, like the reference)
        kth = sv[top_k - 1] if len(sv) >= top_k else sv[-1]
        keep = sv >= kth
        sv = sv[keep]
        si = si[keep]
        # top-p: softmax over survivors, cumulative mass, shifted mask
        m0 = sv[0]
        ex = np.exp((sv - m0).astype(np.float32)).astype(np.float32)
        p = (ex / ex.sum(dtype=np.float32)).astype(np.float32)
        cum = np.cumsum(p, dtype=np.float32)
        rm = np.zeros(len(sv), dtype=bool)
        rm[1:] = cum[:-1] > top_p
        sv = sv[~rm]
        si = si[~rm]
        # min-p on the re-normalized softmax
        ex = np.exp((sv - sv[0]).astype(np.float32)).astype(np.float32)
        p = (ex / ex.sum(dtype=np.float32)).astype(np.float32)
        keep = p >= min_p
        sv = sv[keep]
        si = si[keep]
        adj[r, si] = sv

    # ---- sampling: mirror the reference's jax.random calls exactly ----
    import jax
    import jax.numpy as jnp

    conf = f32(
        f32(f32(f32(1.0) - LE) * f32(SC_LE))
        + f32(f32(f32(1.0) - AE) * f32(SC_AE))
        + f32(f32(f32(1.0) - LV) * f32(SC_LV))
        + f32(f32(f32(1.0) - AV) * f32(SC_AV))
        + f32(AG * f32(SC_AG))
        + f32(IS * f32(SC_IS))
    )

    # The reference can only execute on the CPU backend in this container
    # (argsort is unsupported on trn2), and RBG PRNG bits are backend
    # specific — so draw the samples on CPU to match it bit-for-bit.
    cpu = jax.devices("cpu")[0]
    samples = []
    scores = np.zeros(N_SAMPLES, dtype=np.float32)
    with jax.default_device(cpu):
        adj_j = jnp.asarray(adj)
        key = jax.random.key(42)
        sampled = [
            np.asarray(
                jax.random.categorical(jax.random.fold_in(key, i), adj_j, axis=-1)
            ).astype(np.int32)[:, None]
            for i in range(N_SAMPLES)
        ]
    for i in range(N_SAMPLES):
        s = sampled[i]
        lsm_vals = (
            logits[np.arange(B), s[:, 0]].astype(np.float64) - lse_row
        ).astype(np.float32)
        log_prob = np.sum(lsm_vals, dtype=np.float32)
        samples.append(s)
        scores[i] = f32(log_prob + conf)
    best = int(np.argmax(scores))
    return samples[best], scores


def kernel(logits, attn_scores):
    logits = np.asarray(logits, dtype=np.float32)
    attn_scores = np.asarray(attn_scores, dtype=np.float32)
    results, _ = run_device(logits, attn_scores)
    return _host_finish(logits, results)
